# revision 1
# baseline (speedup 1.0000x reference)
"""Trainium2 Bass kernel for GRU(I=8,H=6) + Linear(6->4) over [B=4096, T=512].

Pure data-parallel over 8 NeuronCores; B/8 = 512 rows per core.

Feature-major on-device layout: the per-core batch of 512 is packed as G=4
groups of 128 batch columns; weights are host-packed into block-diagonal
matrices so one PE pass covers all 4 groups. Every engine AP partition base
is 32-aligned (hardware requirement), so the PSUM gate tile uses 32-row
blocks: [xn @0:24 | hn @32:56 | r @64:88 | z @96:120] (pads zero-filled).

Per timestep t (128 batch columns per group):
  mm1 (PE):   ps[128,128] = Wx.T @ x_t[33,128]    x rows + ones row (biases)
  mm2 (PE):   ps         += Wh.T @ h[25,128]      h rows + ones row
  sig (ACT):  rz[64,128]  = sigmoid(ps[64:128])   r=rz[0:24], z=rz[32:56]
  u   (DVE):  u = rz[0:24] * ps[32:56]            r * hn
  mm_acc(PE): ps[0:24]   += I24.T @ u             xn + r*hn
  tanh(ACT):  n = tanh(ps[0:24])
  d (GPSIMD): d = h[0:24] - n
  e (GPSIMD): e = rz[32:56] * d                   z * (h - n)
  h'  (DVE):  h[0:24] = n + d*z                   new hidden state
  mm3 (PE):   po[16, (t%4)*128:...] = Wlin.T @ h  output projection
  every 4 steps: ACT copy po->SBUF, DMA -> DRAM out

Output leaves the device feature-major [T/4, 16, 512]; host reassembles to
[B, T, 4].
"""

import os
import sys

for _p in ("/opt/trn_rl_repo", "/root/.axon_site/_ro/trn_rl_repo"):
    if os.path.isdir(_p) and _p not in sys.path:
        sys.path.insert(0, _p)

import numpy as np

I, H, O = 8, 6, 4
B, T = 4096, 512
NCORES = 8
BS = B // NCORES        # 512 batch rows per core
G = 4                   # batch groups packed via block-diagonal weights
CB = BS // G            # 128 batch columns per group
GH = G * H              # 24
GI = G * I              # 32
GO = G * O              # 16

_CACHE = {}


def _build_module():
    import concourse.tile as tile
    from concourse import bacc, mybir
    from contextlib import ExitStack

    f32 = mybir.dt.float32
    Sig = mybir.ActivationFunctionType.Sigmoid
    Tanh = mybir.ActivationFunctionType.Tanh
    mult = mybir.AluOpType.mult
    add = mybir.AluOpType.add
    subtract = mybir.AluOpType.subtract

    nc = bacc.Bacc(
        "TRN2",
        target_bir_lowering=False,
        debug=False,
        enable_asserts=False,
        num_devices=NCORES,
    )

    xt_d = nc.dram_tensor("xt", [T, GI + 1, CB], f32, kind="ExternalInput").ap()
    wx_d = nc.dram_tensor("wx", [GI + 1, 128], f32, kind="ExternalInput").ap()
    wh_d = nc.dram_tensor("wh", [GH + 1, 128], f32, kind="ExternalInput").ap()
    wacc_d = nc.dram_tensor("wacc", [GH, GH], f32, kind="ExternalInput").ap()
    wlin_d = nc.dram_tensor("wlin", [GH + 1, GO], f32, kind="ExternalInput").ap()
    hinit_d = nc.dram_tensor("hinit", [GH + 1, CB], f32, kind="ExternalInput").ap()
    out_d = nc.dram_tensor("out", [T // 4, GO, 4 * CB], f32, kind="ExternalOutput").ap()

    with tile.TileContext(nc) as tc, ExitStack() as ctx:
        const = ctx.enter_context(tc.tile_pool(name="const", bufs=1))
        xpool = ctx.enter_context(tc.tile_pool(name="x", bufs=8))
        ps_pool = ctx.enter_context(tc.tile_pool(name="ps", bufs=2, space="PSUM"))
        po_pool = ctx.enter_context(tc.tile_pool(name="po", bufs=2, space="PSUM"))
        rz_pool = ctx.enter_context(tc.tile_pool(name="rz", bufs=3))
        n_pool = ctx.enter_context(tc.tile_pool(name="n", bufs=3))
        u_pool = ctx.enter_context(tc.tile_pool(name="u", bufs=3))
        d_pool = ctx.enter_context(tc.tile_pool(name="d", bufs=3))
        po_sb_pool = ctx.enter_context(tc.tile_pool(name="po_sb", bufs=2))
        hpool = ctx.enter_context(tc.tile_pool(name="h", bufs=1))

        wx_s = const.tile([GI + 1, 128], f32)
        nc.sync.dma_start(wx_s[:], wx_d)
        wh_s = const.tile([GH + 1, 128], f32)
        nc.sync.dma_start(wh_s[:], wh_d)
        wacc_s = const.tile([GH, GH], f32)
        nc.sync.dma_start(wacc_s[:], wacc_d)
        wlin_s = const.tile([GH + 1, GO], f32)
        nc.sync.dma_start(wlin_s[:], wlin_d)

        h_t = hpool.tile([GH + 1, CB], f32)
        nc.sync.dma_start(h_t[:], hinit_d)

        po = None
        for t in range(T):
            x_t = xpool.tile([GI + 1, CB], f32)
            nc.sync.dma_start(x_t[:], xt_d[t, :, :])

            ps = ps_pool.tile([128, CB], f32)
            nc.tensor.matmul(ps[:], wx_s[:], x_t[:], start=True, stop=False)
            nc.tensor.matmul(ps[:], wh_s[:], h_t[:], start=False, stop=False)

            # psum rows 64:128 hold [z @64:88 | r @96:120]; after the copy
            # z = rz[0:24] (base 0, matches d), r = rz[32:56] (base 32, matches hn)
            rz = rz_pool.tile([64, CB], f32)
            nc.scalar.activation(rz[:], ps[64:128, :], Sig)

            u = u_pool.tile([GH, CB], f32)
            nc.vector.tensor_tensor(out=u[:], in0=rz[32 : 32 + GH, :], in1=ps[32 : 32 + GH, :], op=mult)

            nc.tensor.matmul(ps[0:GH, :], wacc_s[:], u[:], start=False, stop=True)

            n_ = n_pool.tile([GH, CB], f32)
            nc.scalar.activation(n_[:], ps[0:GH, :], Tanh)

            d_ = d_pool.tile([GH, CB], f32)
            nc.gpsimd.tensor_tensor(out=d_[:], in0=h_t[0:GH, :], in1=n_[:], op=subtract)

            e_ = d_pool.tile([GH, CB], f32, tag="e")
            nc.gpsimd.tensor_tensor(out=e_[:], in0=rz[0:GH, :], in1=d_[:], op=mult)

            nc.vector.tensor_tensor(out=h_t[0:GH, :], in0=n_[:], in1=e_[:], op=add)

            tt = t % 4
            if tt == 0:
                po = po_pool.tile([GO, 4 * CB], f32)
            nc.tensor.matmul(
                po[:, tt * CB : (tt + 1) * CB], wlin_s[:], h_t[:], start=True, stop=True
            )
            if tt == 3:
                po_sb = po_sb_pool.tile([GO, 4 * CB], f32)
                nc.scalar.copy(po_sb[:], po[:])
                nc.sync.dma_start(out_d[t // 4, :, :], po_sb[:])

    nc.compile()
    return nc


def _pack_weights(W_ih, W_hh, b_ih, b_hh, W_lin, b_lin):
    # psum row blocks (32-aligned): xn @0, hn @32, r @64, z @96
    wx = np.zeros((GI + 1, 128), np.float32)
    wh = np.zeros((GH + 1, 128), np.float32)
    wlin = np.zeros((GH + 1, GO), np.float32)
    for g in range(G):
        sl_x = slice(g * I, (g + 1) * I)
        sl_h = slice(g * H, (g + 1) * H)
        # xn block: x weights + b_ih[n] on x ones-row
        wx[sl_x, 0 + g * H : 0 + (g + 1) * H] = W_ih[12:18].T
        wx[GI, 0 + g * H : 0 + (g + 1) * H] = b_ih[12:18]
        # hn block: h weights + b_hh[n] on h ones-row
        wh[sl_h, 32 + g * H : 32 + (g + 1) * H] = W_hh[12:18].T
        wh[GH, 32 + g * H : 32 + (g + 1) * H] = b_hh[12:18]
        # z block @64: both weights, biases on x ones-row
        wx[sl_x, 64 + g * H : 64 + (g + 1) * H] = W_ih[6:12].T
        wx[GI, 64 + g * H : 64 + (g + 1) * H] = b_ih[6:12] + b_hh[6:12]
        wh[sl_h, 64 + g * H : 64 + (g + 1) * H] = W_hh[6:12].T
        # r block @96
        wx[sl_x, 96 + g * H : 96 + (g + 1) * H] = W_ih[0:6].T
        wx[GI, 96 + g * H : 96 + (g + 1) * H] = b_ih[0:6] + b_hh[0:6]
        wh[sl_h, 96 + g * H : 96 + (g + 1) * H] = W_hh[0:6].T
        # linear projection
        wlin[sl_h, g * O : (g + 1) * O] = W_lin.T
        wlin[GH, g * O : (g + 1) * O] = b_lin
    wacc = np.eye(GH, dtype=np.float32)
    return wx, wh, wacc, wlin


def _run(inputs, trace=False):
    from concourse.bass_utils import run_bass_kernel_spmd

    x = np.ascontiguousarray(np.asarray(inputs["x"], dtype=np.float32))
    W_ih = np.asarray(inputs["W_ih"], np.float32)
    W_hh = np.asarray(inputs["W_hh"], np.float32)
    b_ih = np.asarray(inputs["b_ih"], np.float32)
    b_hh = np.asarray(inputs["b_hh"], np.float32)
    W_lin = np.asarray(inputs["W_lin"], np.float32)
    b_lin = np.asarray(inputs["b_lin"], np.float32)

    if "nc" not in _CACHE:
        _CACHE["nc"] = _build_module()
    nc = _CACHE["nc"]

    wx, wh, wacc, wlin = _pack_weights(W_ih, W_hh, b_ih, b_hh, W_lin, b_lin)
    hinit = np.zeros((GH + 1, CB), np.float32)
    hinit[GH, :] = 1.0

    in_maps = []
    for c in range(NCORES):
        xc = x[c * BS : (c + 1) * BS]                     # [512, 512, 8]
        xt = np.ones((T, GI + 1, CB), np.float32)
        xt[:, :GI, :] = xc.reshape(G, CB, T, I).transpose(2, 0, 3, 1).reshape(T, GI, CB)
        in_maps.append(
            {"xt": xt, "wx": wx, "wh": wh, "wacc": wacc, "wlin": wlin, "hinit": hinit}
        )

    res = run_bass_kernel_spmd(
        nc, in_maps, core_ids=list(range(NCORES)), trace=trace
    )

    outs = []
    for c in range(NCORES):
        a = res.results[c]["out"]                        # [T/4, 16, 512]
        a = a.reshape(T // 4, G, O, 4, CB)               # [t4, g, o, tt, b]
        a = a.transpose(1, 4, 0, 3, 2)                   # [g, b, t4, tt, o]
        outs.append(a.reshape(BS, T, O))
    full = np.concatenate(outs, axis=0)
    return full, res


def kernel(**inputs) -> np.ndarray:
    out, _ = _run(inputs, trace=False)
    return out


def kernel_profiled(inputs):
    """Returns (output, BassKernelResults-with-trace)."""
    return _run(inputs, trace=True)



# revision 8
# speedup vs baseline: 1.1795x; 1.1795x over previous
"""Trainium2 Bass kernel for GRU(I=8,H=6) + Linear(6->4) over [B=4096, T=512].

Pure data-parallel over 8 NeuronCores; B/8 = 512 rows per core.

v2: two interleaved independent chains per core (2 batch groups of 128
columns each), fp16 matmuls (1 cyc/row vs fp32's 4), negated z-weights so
one sigmoid yields zb = 1-z directly, and engine rebalance so the serial
scan's latency is hidden by ping-ponging the two chains:

  per chain, PSUM gate blocks (32-aligned): [pn_x @0:12 | pn_h @32:44 |
  zb @64:76 | r @96:108], 128 batch columns.

  mm1 (PE):   ps  = Wx.T @ x_t[16,128]        (prefetched, off chain)
  mm2 (PE):   ps += Wh.T @ h[13,128]          h rows + ones row (biases)
  sig (ACT):  rz[64,128] = sigmoid(ps[64:128])  zb=rz[0:12], r=rz[32:44]
  u  (POOL):  u = r * pn_h
  acc (PE):   ps[0:12] += I12.T @ u            pn_x + r*pn_h
  tanh (ACT): n = tanh(ps[0:12])
  v   (DVE):  v = n - h
  w   (DVE):  w = zb * v
  h'  (DVE):  h += w                           = z*h + (1-z)*n
  mm3 (PE):   po[8, tt*128:] = Wlin.T @ h
  every 4 steps: copy po->SBUF fp16 (DVE for chain0, POOL for chain1),
  DMA -> DRAM out.

Output leaves the device as [T/4, 8, 512] fp16 per chain; host reassembles.
"""

import os
import sys

for _p in ("/opt/trn_rl_repo", "/root/.axon_site/_ro/trn_rl_repo"):
    if os.path.isdir(_p) and _p not in sys.path:
        sys.path.insert(0, _p)

import numpy as np

I, H, O = 8, 6, 4
B, T = 4096, 512
NCORES = 8
BS = B // NCORES        # 512 batch rows per core
NC_CHAINS = 2           # independent interleaved chains per core
GPC = 2                 # batch groups per chain
CB = 128                # batch columns per group
XR = GPC * I            # 16 x rows per chain
HR = GPC * H            # 12 h rows per chain
OR = GPC * O            # 8 out rows per chain

_CACHE = {}


def _build_module():
    import concourse.tile as tile
    from concourse import bacc, mybir
    from contextlib import ExitStack

    f16 = mybir.dt.float16
    Sig = mybir.ActivationFunctionType.Sigmoid
    Tanh = mybir.ActivationFunctionType.Tanh
    mult = mybir.AluOpType.mult
    add = mybir.AluOpType.add
    subtract = mybir.AluOpType.subtract

    nc = bacc.Bacc(
        "TRN2",
        target_bir_lowering=False,
        debug=False,
        enable_asserts=False,
        num_devices=NCORES,
    )

    xt_d = [
        nc.dram_tensor(f"xt{c}", [T, XR, CB], f16, kind="ExternalInput").ap()
        for c in range(NC_CHAINS)
    ]
    wx_d = nc.dram_tensor("wx", [XR, 128], f16, kind="ExternalInput").ap()
    wh_d = nc.dram_tensor("wh", [HR + 1, 128], f16, kind="ExternalInput").ap()
    wacc_d = nc.dram_tensor("wacc", [HR, HR], f16, kind="ExternalInput").ap()
    wlin_d = nc.dram_tensor("wlin", [HR + 1, OR], f16, kind="ExternalInput").ap()
    hinit_d = nc.dram_tensor("hinit", [HR + 1, CB], f16, kind="ExternalInput").ap()
    out_d = [
        nc.dram_tensor(f"out{c}", [T // 4, OR, 4 * CB], f16, kind="ExternalOutput").ap()
        for c in range(NC_CHAINS)
    ]

    CH = range(NC_CHAINS)

    with tile.TileContext(nc) as tc, ExitStack() as ctx:
        const = ctx.enter_context(tc.tile_pool(name="const", bufs=1))
        xpool = [
            ctx.enter_context(tc.tile_pool(name=f"x{c}", bufs=6)) for c in CH
        ]
        ps_pool = [
            ctx.enter_context(tc.tile_pool(name=f"ps{c}", bufs=2, space="PSUM"))
            for c in CH
        ]
        po_pool = [
            ctx.enter_context(tc.tile_pool(name=f"po{c}", bufs=2, space="PSUM"))
            for c in CH
        ]
        rz_pool = [
            ctx.enter_context(tc.tile_pool(name=f"rz{c}", bufs=3)) for c in CH
        ]
        u_pool = [
            ctx.enter_context(tc.tile_pool(name=f"u{c}", bufs=3)) for c in CH
        ]
        n_pool = [
            ctx.enter_context(tc.tile_pool(name=f"n{c}", bufs=3)) for c in CH
        ]
        v_pool = [
            ctx.enter_context(tc.tile_pool(name=f"v{c}", bufs=3)) for c in CH
        ]
        w_pool = [
            ctx.enter_context(tc.tile_pool(name=f"w{c}", bufs=3)) for c in CH
        ]
        po_sb_pool = [
            ctx.enter_context(tc.tile_pool(name=f"posb{c}", bufs=2)) for c in CH
        ]
        hpool = [
            ctx.enter_context(tc.tile_pool(name=f"h{c}", bufs=1)) for c in CH
        ]

        wx_s = const.tile([XR, 128], f16)
        nc.sync.dma_start(wx_s[:], wx_d)
        wh_s = const.tile([HR + 1, 128], f16)
        nc.sync.dma_start(wh_s[:], wh_d)
        wacc_s = const.tile([HR, HR], f16)
        nc.sync.dma_start(wacc_s[:], wacc_d)
        wlin_s = const.tile([HR + 1, OR], f16)
        nc.sync.dma_start(wlin_s[:], wlin_d)

        h_t = []
        for c in CH:
            h = hpool[c].tile([HR + 1, CB], f16)
            nc.sync.dma_start(h[:], hinit_d)
            h_t.append(h)

        # prologue: prefetch x(0..1), mm1(0)
        x_t = {}
        ps = {}
        po = [None, None]
        for tp in (0, 1):
            for c in CH:
                xt = xpool[c].tile([XR, CB], f16)
                nc.gpsimd.dma_start(xt[:], xt_d[c][tp, :, :])
                x_t[(tp, c)] = xt
        for c in CH:
            p = ps_pool[c].tile([128, CB], mybir.dt.float32)
            nc.tensor.matmul(p[:], wx_s[:], x_t[(0, c)][:], start=True, stop=False)
            ps[(0, c)] = p

        for t in range(T):
            tt = t % 4
            # prefetch next x + mm1(t+1)
            if t + 2 < T:
                for c in CH:
                    xt = xpool[c].tile([XR, CB], f16)
                    nc.gpsimd.dma_start(xt[:], xt_d[c][t + 2, :, :])
                    x_t[(t + 2, c)] = xt
            if t + 1 < T:
                for c in CH:
                    p = ps_pool[c].tile([128, CB], mybir.dt.float32)
                    nc.tensor.matmul(
                        p[:], wx_s[:], x_t[(t + 1, c)][:], start=True, stop=False
                    )
                    ps[(t + 1, c)] = p
                    del x_t[(t + 1, c)]

            cur = [ps[(t, c)] for c in CH]
            for c in CH:
                nc.tensor.matmul(cur[c][:], wh_s[:], h_t[c][:], start=False, stop=False)

            rz = []
            for c in CH:
                r = rz_pool[c].tile([64, CB], f16)
                nc.scalar.activation(r[:], cur[c][64:128, :], Sig)
                rz.append(r)

            u = []
            for c in CH:
                uu = u_pool[c].tile([HR, CB], f16)
                nc.vector.tensor_tensor(
                    out=uu[:], in0=rz[c][32 : 32 + HR, :], in1=cur[c][32 : 32 + HR, :], op=mult
                )
                u.append(uu)

            for c in CH:
                nc.tensor.matmul(cur[c][0:HR, :], wacc_s[:], u[c][:], start=False, stop=True)

            n_ = []
            for c in CH:
                nn_ = n_pool[c].tile([HR, CB], f16)
                nc.scalar.activation(nn_[:], cur[c][0:HR, :], Tanh)
                n_.append(nn_)

            v_ = []
            for c in CH:
                vv = v_pool[c].tile([HR, CB], f16)
                nc.gpsimd.tensor_tensor(out=vv[:], in0=n_[c][:], in1=h_t[c][0:HR, :], op=subtract)
                v_.append(vv)

            w_ = []
            for c in CH:
                ww = w_pool[c].tile([HR, CB], f16)
                nc.vector.tensor_tensor(out=ww[:], in0=rz[c][0:HR, :], in1=v_[c][:], op=mult)
                w_.append(ww)

            for c in CH:
                nc.vector.tensor_tensor(
                    out=h_t[c][0:HR, :], in0=h_t[c][0:HR, :], in1=w_[c][:], op=add
                )

            for c in CH:
                if tt == 0:
                    po[c] = po_pool[c].tile(
                        [OR, 4 * CB], mybir.dt.float32, name=f"po{c}"
                    )
                nc.tensor.matmul(
                    po[c][:, tt * CB : (tt + 1) * CB], wlin_s[:], h_t[c][:],
                    start=True, stop=True,
                )

            if tt == 3:
                for c in CH:
                    po_sb = po_sb_pool[c].tile([OR, 4 * CB], f16)
                    nc.vector.tensor_copy(po_sb[:], po[c][:])
                    nc.sync.dma_start(out_d[c][t // 4, :, :], po_sb[:])

            del ps[(t, 0)], ps[(t, 1)]

    nc.compile()
    return nc


def _pack_weights(W_ih, W_hh, b_ih, b_hh, W_lin, b_lin):
    # psum col blocks (32-aligned): pn_x @0, pn_h @32, zb @64 (negated), r @96
    wx = np.zeros((XR, 128), np.float32)
    wh = np.zeros((HR + 1, 128), np.float32)
    wlin = np.zeros((HR + 1, OR), np.float32)
    for g in range(GPC):
        sx = slice(g * I, (g + 1) * I)
        sh = slice(g * H, (g + 1) * H)
        # pn_x block: x n-weights; b_in on wh ones row
        wx[sx, 0 + g * H : 0 + (g + 1) * H] = W_ih[12:18].T
        wh[HR, 0 + g * H : 0 + (g + 1) * H] = b_ih[12:18]
        # pn_h block: h n-weights + b_hn on ones row
        wh[sh, 32 + g * H : 32 + (g + 1) * H] = W_hh[12:18].T
        wh[HR, 32 + g * H : 32 + (g + 1) * H] = b_hh[12:18]
        # zb block @64: NEGATED z pre-activation -> sigmoid gives 1-z
        wx[sx, 64 + g * H : 64 + (g + 1) * H] = -W_ih[6:12].T
        wh[sh, 64 + g * H : 64 + (g + 1) * H] = -W_hh[6:12].T
        wh[HR, 64 + g * H : 64 + (g + 1) * H] = -(b_ih[6:12] + b_hh[6:12])
        # r block @96
        wx[sx, 96 + g * H : 96 + (g + 1) * H] = W_ih[0:6].T
        wh[sh, 96 + g * H : 96 + (g + 1) * H] = W_hh[0:6].T
        wh[HR, 96 + g * H : 96 + (g + 1) * H] = b_ih[0:6] + b_hh[0:6]
        # linear projection
        wlin[sh, g * O : (g + 1) * O] = W_lin.T
        wlin[HR, g * O : (g + 1) * O] = b_lin
    wacc = np.eye(HR, dtype=np.float32)
    return (
        wx.astype(np.float16),
        wh.astype(np.float16),
        wacc.astype(np.float16),
        wlin.astype(np.float16),
    )


def _run(inputs, trace=False):
    from concourse.bass_utils import run_bass_kernel_spmd

    x = np.ascontiguousarray(np.asarray(inputs["x"], dtype=np.float32))
    W_ih = np.asarray(inputs["W_ih"], np.float32)
    W_hh = np.asarray(inputs["W_hh"], np.float32)
    b_ih = np.asarray(inputs["b_ih"], np.float32)
    b_hh = np.asarray(inputs["b_hh"], np.float32)
    W_lin = np.asarray(inputs["W_lin"], np.float32)
    b_lin = np.asarray(inputs["b_lin"], np.float32)

    if "nc" not in _CACHE:
        _CACHE["nc"] = _build_module()
    nc = _CACHE["nc"]

    wx, wh, wacc, wlin = _pack_weights(W_ih, W_hh, b_ih, b_hh, W_lin, b_lin)
    hinit = np.zeros((HR + 1, CB), np.float16)
    hinit[HR, :] = 1.0

    in_maps = []
    for core in range(NCORES):
        xc = x[core * BS : (core + 1) * BS]              # [512, 512, 8]
        im = {"wx": wx, "wh": wh, "wacc": wacc, "wlin": wlin, "hinit": hinit}
        for c in range(NC_CHAINS):
            xcc = xc[c * GPC * CB : (c + 1) * GPC * CB]  # [256, T, I]
            # xt[t, g*I+i, b] = xcc[g*CB+b, t, i]
            xt = (
                xcc.reshape(GPC, CB, T, I)
                .transpose(2, 0, 3, 1)
                .reshape(T, XR, CB)
                .astype(np.float16)
            )
            im[f"xt{c}"] = np.ascontiguousarray(xt)
        in_maps.append(im)

    res = run_bass_kernel_spmd(
        nc, in_maps, core_ids=list(range(NCORES)), trace=trace
    )

    outs = []
    for core in range(NCORES):
        for c in range(NC_CHAINS):
            a = res.results[core][f"out{c}"]             # [T/4, 8, 512] fp16
            a = a.astype(np.float32)
            a = a.reshape(T // 4, GPC, O, 4, CB)         # [t4, g, o, tt, b]
            a = a.transpose(1, 4, 0, 3, 2)               # [g, b, t4, tt, o]
            outs.append(a.reshape(GPC * CB, T, O))
    full = np.concatenate(outs, axis=0)
    return full, res


def kernel(**inputs) -> np.ndarray:
    out, _ = _run(inputs, trace=False)
    return out


def kernel_profiled(inputs):
    """Returns (output, BassKernelResults-with-trace)."""
    return _run(inputs, trace=True)
